# revision 1
# baseline (speedup 1.0000x reference)
"""Weighted-KNN (retrieval_knn) Trainium2 kernel.

Math (per query c, over N anchors):
    sq[n]   = ||c - p_n||^2 / (w_n^2 + eps)
    top-8 smallest sq -> softmax(-sq_k / TEMP) -> weighted sum of features.

Device strategy (per core, data-parallel over B across 8 cores):
  * y[q, n] = -sq[q,n]/TEMP computed on TensorE as a rank-5 inner product
    over centered coordinates (c' = c - 0.5, p' = p - 0.5):
        h_q = [||c'_q||^2, c'_q0, c'_q1, c'_q2, 1]
        g_n = (-1/TEMP) * inv_n * [1, -2p'_n0, -2p'_n1, -2p'_n2, ||p'_n||^2]
    The anchor axis is split into 4 groups handled by 4 concurrent
    row-tiled matmuls (tile_position=(32m, 0)); each group's 5 G rows and
    5 H rows live on disjoint 32-aligned partition lanes, so the four
    fp32 streams overlap on the PE array.
  * Packed top-8 trick: the PSUM->SBUF copy writes y as bf16 into the HIGH
    halves of persistent fp32 words whose LOW halves hold the (half-
    relative) anchor index, initialized once.  fp32 `max` (DVE top-8) on
    the packed words yields value ordering AND the index in one pass --
    no max_index scan.
  * Top-8 of each N/2 half (the union provably contains the true top-8:
    any true top-8 member has at most 7 better anchors anywhere).
  * The 16 candidates are re-scored EXACTLY from gathered [p', -inv/TEMP]
    rows (direct differences, no cancellation); top-8 selection + softmax
    run on the exact scores, so bf16/matmul rounding only perturbs
    candidates around global rank ~16 whose weights are negligible.
  * Feature rows fetched with gpsimd dma_gather; masked softmax-weighted
    sum on VectorE.
"""

import sys

if "/opt/trn_rl_repo" not in sys.path:
    sys.path.insert(0, "/opt/trn_rl_repo")

import numpy as np

import concourse.bacc as bacc
import concourse.bass as bass
import concourse.mybir as mybir
from concourse.bass import ts
from concourse.bass_utils import run_bass_kernel_spmd
from concourse.tile import TileContext

B, N, D, F = 65536, 16384, 3, 64
K = 8
BANDWIDTH = 0.05
TEMP = 2.0 * BANDWIDTH * BANDWIDTH  # 0.005
INV_TEMP = 1.0 / TEMP  # 200.0
EPS = 1e-8
NCORES = 8
Q = B // NCORES  # 8192 queries per core
P = 128
NT = Q // P  # 64 query tiles per core
CH = 512  # matmul free-dim chunk
NG = 4  # row-tiled matmul groups
NGN = N // NG  # 4096 anchors per group
NCHG = NGN // CH  # 8 chunk-steps
NH = N // NG  # 4096 anchors per quarter(=group)
NC = NG * K  # 32 candidates per query
NPK = 6  # packed-y buffer rotation depth
LOOP = 1  # in-NEFF repetitions of the whole tile loop (benchmarking)
STAGE = 99  # bench bisect: 1=mm+max8, 2=+perm/idxw, 3=+gathers, 99=full

FP = mybir.dt.float32
BF = mybir.dt.bfloat16
I32 = mybir.dt.int32


def _build_nc():
    nc = bacc.Bacc("TRN2", num_swdge_queues=2)
    coords = nc.declare_dram_parameter("coords", [Q, D], FP, isOutput=False)
    positions = nc.declare_dram_parameter("positions", [N, D], FP, isOutput=False)
    weights = nc.declare_dram_parameter("weights", [N], FP, isOutput=False)
    features = nc.declare_dram_parameter("features", [N, F], FP, isOutput=False)
    ident_in = nc.declare_dram_parameter("ident", [P, P], FP, isOutput=False)
    perm_in = nc.declare_dram_parameter("perm", [P, 8, P], FP, isOutput=False)
    pkinit_in = nc.declare_dram_parameter("pkinit", [P, NH], I32, isOutput=False)
    out = nc.declare_dram_parameter("out", [Q, F], FP, isOutput=True)

    # combined gather table: row n = [features(64) | p'_0 p'_1 p'_2 -inv/TEMP | pad]
    comb_hbm = nc.dram_tensor("comb_stage", [N, 2 * F], FP)

    with TileContext(nc) as tc:
        with (
            tc.tile_pool(name="const", bufs=1) as cpool,
            nc.gpsimd.register("nidx") as nidx_reg,
        ):
            nc.gpsimd.reg_mov(nidx_reg, P * K)

            ident = cpool.tile([P, P], FP)
            nc.sync.dma_start(ident[:], ident_in[:])
            pconst = cpool.tile([P, 8, P], FP)
            nc.sync.dma_start(pconst[:], perm_in[:])

            # G4[32m + r, j] = g_r[m*4096 + j]  (4 groups on partition lanes)
            G4 = cpool.tile([P, NGN], FP)

            # persistent packed-y buffers; low halves = half-relative idx
            pk = [
                cpool.tile([P, NH], FP, name=f"pk{i}", tag=f"pk{i}")
                for i in range(NPK)
            ]
            for i in range(NPK):
                nc.sync.dma_start(pk[i][:].bitcast(I32), pkinit_in[:])

            # ---------------- prep: build G and the rescore table ----------------
            with tc.tile_pool(name="prep", bufs=2) as pp:
                # anchors laid out n = 128*p + j
                pos_sb = pp.tile([P, P, D], FP)
                nc.sync.dma_start(
                    pos_sb[:], positions[:].rearrange("(p j) d -> p j d", p=P)
                )
                # center: p' = p - 0.5
                nc.vector.tensor_scalar_add(pos_sb[:], pos_sb[:], -0.5)
                w_sb = pp.tile([P, P], FP)
                nc.sync.dma_start(w_sb[:], weights[:].rearrange("(p j) -> p j", p=P))

                inv = pp.tile([P, P], FP)
                nc.vector.tensor_mul(inv[:], w_sb[:], w_sb[:])
                nc.vector.tensor_scalar_add(inv[:], inv[:], EPS)
                nc.vector.reciprocal(inv[:], inv[:])

                g0 = pp.tile([P, P], FP)
                nc.vector.tensor_scalar_mul(g0[:], inv[:], -INV_TEMP)

                gd = [
                    pp.tile([P, P], FP, tag=f"g{d + 1}", name=f"g{d + 1}")
                    for d in range(D)
                ]
                for d in range(D):
                    nc.vector.tensor_mul(gd[d][:], inv[:], pos_sb[:, :, d])
                    nc.vector.tensor_scalar_mul(gd[d][:], gd[d][:], 2.0 * INV_TEMP)

                pp2 = pp.tile([P, P], FP)
                tmp = pp.tile([P, P], FP)
                nc.vector.tensor_mul(pp2[:], pos_sb[:, :, 0], pos_sb[:, :, 0])
                nc.vector.tensor_mul(tmp[:], pos_sb[:, :, 1], pos_sb[:, :, 1])
                nc.vector.tensor_add(pp2[:], pp2[:], tmp[:])
                nc.vector.tensor_mul(tmp[:], pos_sb[:, :, 2], pos_sb[:, :, 2])
                nc.vector.tensor_add(pp2[:], pp2[:], tmp[:])
                g4c = pp.tile([P, P], FP)
                nc.vector.tensor_mul(g4c[:], g0[:], pp2[:])

                # scatter [128, 128] component tiles into G4 group lanes:
                # comp_r partitions [32m, 32m+32) hold n in [4096m, 4096m+4096)
                for r, comp in enumerate([g0, gd[0], gd[1], gd[2], g4c]):
                    for m in range(NG):
                        src = comp[32 * m : 32 * (m + 1), :]
                        dst = bass.AP(
                            G4[:].tensor,
                            (32 * m + r) * NGN,
                            [[NGN, 1], [P, 32], [1, P]],
                        )
                        nc.sync.dma_start(dst, src)

                # rescore table: interleave [p'0, p'1, p'2, g0] per anchor,
                # DMA'd (chunked) into the first 16 bytes of each 256B row.
                pwt = pp.tile([P, P, 4], FP)
                for f, comp in enumerate(
                    [pos_sb[:, :, 0], pos_sb[:, :, 1], pos_sb[:, :, 2], g0[:]]
                ):
                    nc.vector.tensor_copy(pwt[:, :, f], comp)
                pw_rows = comb_hbm[:, F : F + 4].rearrange("(p j) f -> p j f", p=P)
                for ck in range(8):
                    nc.sync.dma_start(
                        pw_rows[ts(ck, 16), :, :], pwt[ts(ck, 16), :, :]
                    )
                # features -> comb rows (HBM->HBM, chunked)
                feat_rows = comb_hbm[:, 0:F].rearrange("(a n) f -> a n f", a=8)
                src_rows = features[:].rearrange("(a n) f -> a n f", a=8)
                for ck in range(8):
                    nc.sync.dma_start(feat_rows[ck, :, :], src_rows[ck, :, :])

            # ---------------- main loop over query tiles ----------------
            with (
                tc.tile_pool(name="mm_ps", bufs=6, space="PSUM") as pspool,
                tc.tile_pool(name="ht_ps", bufs=2, space="PSUM") as htpool,
                tc.tile_pool(name="hs", bufs=4) as hpool,
                tc.tile_pool(name="sm", bufs=6) as sm,
                tc.tile_pool(name="g8", bufs=3) as gpool,
            ):
                for tl in range(NT * LOOP):
                    t = tl % NT
                    # --- per-tile H build, replicated to 4 group lanes ---
                    ct = hpool.tile([P, D], FP, tag="ct")
                    nc.sync.dma_start(ct[:], coords[ts(t, P), :])
                    nc.vector.tensor_scalar_add(ct[:], ct[:], -0.5)
                    nct = hpool.tile([P, D], FP, tag="nct")
                    nc.vector.tensor_scalar_mul(nct[:], ct[:], -1.0)
                    cc = hpool.tile([P, D], FP, tag="cc")
                    nc.vector.tensor_mul(cc[:], ct[:], ct[:])
                    hsrc = hpool.tile([P, 5], FP, tag="hsrc")
                    nc.vector.reduce_sum(
                        out=hsrc[:, 0:1], in_=cc[:], axis=mybir.AxisListType.X
                    )
                    nc.vector.tensor_copy(hsrc[:, 1:4], ct[:])
                    nc.vector.memset(hsrc[:, 4:5], 1.0)
                    psT = htpool.tile([P, P], FP, tag="htmp")
                    nc.tensor.transpose(psT[:5, :], hsrc[:], ident[:])
                    hT4 = hpool.tile([P, P], FP, tag="hT4")
                    for m in range(NG):
                        nc.scalar.copy(hT4[32 * m : 32 * m + 5, :], psT[:5, :])

                    # --- distances: 4 concurrent row-tiled matmuls/step ---
                    pkb = [pk[(NG * tl + m) % NPK] for m in range(NG)]
                    for c in range(NCHG):
                        for m in range(NG):
                            ps = pspool.tile(
                                [P, CH], FP, tag="ps", name=f"ps{tl}_{c}_{m}"
                            )
                            nc.tensor.matmul(
                                ps[:],
                                hT4[32 * m : 32 * m + 5, :],
                                G4[32 * m : 32 * m + 5, ts(c, CH)],
                                start=True,
                                stop=True,
                                tile_position=(32 * m, 0),
                            )
                            # bf16 y into high halves of group m's words
                            hi = bass.AP(
                                pkb[m][:].bitcast(BF).tensor,
                                2 * CH * c + 1,
                                [[2 * NH, P], [2, CH]],
                            )
                            nc.scalar.copy(hi, ps[:])

                    # --- packed top-8 per quarter; extract indices ---
                    idx32 = sm.tile([P, NC], I32, tag="idx32")
                    for m in range(NG):
                        v8p = sm.tile([P, K], FP, tag="v8p", name=f"v8p_{tl}_{m}")
                        nc.vector.max(v8p[:], pkb[m][:])
                        nc.vector.tensor_scalar(
                            idx32[:, ts(m, K)],
                            v8p[:].bitcast(I32),
                            65535,
                            None,
                            op0=mybir.AluOpType.bitwise_and,
                        )
                        if m:
                            nc.vector.tensor_scalar_add(
                                idx32[:, ts(m, K)], idx32[:, ts(m, K)], float(m * NH)
                            )

                    if STAGE == 1:
                        dump = sm.tile([P, F], FP, tag="dump", name=f"dump{tl}")
                        nc.vector.tensor_copy(dump[:, 0:NC], idx32[:])
                        nc.sync.dma_start(out[ts(t, P), :], dump[:])
                        continue

                    # --- wrapped int16 idx layout for dma_gather:
                    # idxw[16g+p, 8k+u] = idx[16u+p, k] via 8 permutation
                    # matmuls (perm_u[q, p'] = 1 iff q == 16u + p'%16).
                    idxf = sm.tile([P, NC], FP, tag="idxf")
                    nc.vector.tensor_copy(idxf[:], idx32[:])
                    psI = htpool.tile([P, 8, NC], FP, tag="htmp", name=f"psI_{tl}")
                    for u in range(8):
                        nc.tensor.matmul(
                            psI[:, u, :],
                            pconst[:, u, :],
                            idxf[:],
                            start=True,
                            stop=True,
                        )
                    idxw = sm.tile([P, NC * 8], mybir.dt.int16, tag="idxw")
                    idxw_uk = bass.AP(
                        idxw[:].tensor, 0, [[NC * 8, P], [1, 8], [8, NC]]
                    )
                    nc.vector.tensor_copy(idxw_uk, psI[:])

                    if STAGE == 2:
                        dump = sm.tile([P, F], FP, tag="dump", name=f"dump{tl}")
                        nc.vector.tensor_copy(dump[:], idxw[:, 0:128].bitcast(FP))
                        nc.sync.dma_start(out[ts(t, P), :], dump[:])
                        continue

                    # --- gather candidate rescore rows + feature rows ---
                    cg = gpool.tile([P, NC, 2 * F], FP, tag="cg")
                    for m in range(NG):
                        isl = idxw[:, m * K * 8 : (m + 1) * K * 8]
                        nc.gpsimd.dma_gather(
                            cg[:, m * K : (m + 1) * K, :],
                            comb_hbm[:],
                            isl,
                            P * K,
                            nidx_reg,
                            2 * F,
                            queue_num=m % 2,
                        )
                    g32 = cg[:, :, 0:F]
                    pwg = cg[:, :, F : 2 * F]

                    if STAGE == 3:
                        dump = sm.tile([P, F], FP, tag="dump", name=f"dump{tl}")
                        nc.vector.tensor_add(dump[:], cg[:, 0, 0:F], cg[:, 0, F:2*F])
                        nc.sync.dma_start(out[ts(t, P), :], dump[:])
                        continue

                    # --- exact rescore: y32 = sum_d (p'_d - c'_d)^2 * g0 ---
                    sqd = [
                        sm.tile([P, NC], FP, tag=f"sqd{d}", name=f"sqd{d}")
                        for d in range(D)
                    ]
                    for d in range(D):
                        nc.scalar.activation(
                            sqd[d][:],
                            pwg[:, :, d],
                            mybir.ActivationFunctionType.Square,
                            bias=nct[:, d : d + 1],
                            scale=1.0,
                        )
                    nc.vector.tensor_add(sqd[0][:], sqd[0][:], sqd[1][:])
                    nc.vector.tensor_add(sqd[0][:], sqd[0][:], sqd[2][:])
                    y16 = sm.tile([P, NC], FP, tag="y16")
                    nc.vector.tensor_mul(y16[:], sqd[0][:], pwg[:, :, 3])

                    # --- exact top-8 + masked softmax over 16 candidates ---
                    v8x = sm.tile([P, K], FP, tag="v8x")
                    nc.vector.max(v8x[:], y16[:])
                    nv1 = sm.tile([P, 1], FP, tag="nv1")
                    nc.vector.tensor_scalar_mul(nv1[:], v8x[:, 0:1], -1.0)
                    e16 = sm.tile([P, NC], FP, tag="e16")
                    nc.scalar.activation(
                        e16[:],
                        y16[:],
                        mybir.ActivationFunctionType.Exp,
                        bias=nv1[:],
                        scale=1.0,
                    )
                    m16 = sm.tile([P, NC], FP, tag="m16")
                    nc.vector.tensor_scalar(
                        m16[:],
                        y16[:],
                        v8x[:, K - 1 : K],
                        None,
                        op0=mybir.AluOpType.is_ge,
                    )
                    ew = sm.tile([P, NC], FP, tag="ew")
                    nc.vector.tensor_mul(ew[:], e16[:], m16[:])
                    ssum = sm.tile([P, 1], FP, tag="ssum")
                    nc.vector.reduce_sum(
                        out=ssum[:], in_=ew[:], axis=mybir.AxisListType.X
                    )
                    rs = sm.tile([P, 1], FP, tag="rs")
                    nc.vector.reciprocal(rs[:], ssum[:])

                    # --- weighted sum of candidate features ---
                    nc.vector.tensor_mul(
                        g32, g32, ew[:].to_broadcast([P, NC, F])
                    )
                    half = NC
                    while half > 1:
                        half //= 2
                        nc.vector.tensor_add(
                            cg[:, 0:half, 0:F],
                            cg[:, 0:half, 0:F],
                            cg[:, half : 2 * half, 0:F],
                        )
                    ot = gpool.tile([P, F], FP, tag="ot")
                    nc.vector.tensor_scalar_mul(ot[:], cg[:, 0, 0:F], rs[:])

                    nc.sync.dma_start(out[ts(t, P), :], ot[:])

    nc.compile()
    return nc


_NC = None
LAST_RESULT = None


def _host_consts():
    ident = np.eye(P, dtype=np.float32)
    perm = np.zeros((P, 8, P), dtype=np.float32)
    for u in range(8):
        for p16 in range(16):
            perm[16 * u + p16, u, p16::16] = 1.0
    pkinit = np.tile(np.arange(NH, dtype=np.int32), (P, 1))
    return ident, perm, pkinit


def kernel(coords, positions, weights, features):
    global _NC, LAST_RESULT
    import os

    if _NC is None:
        _NC = _build_nc()

    coords = np.ascontiguousarray(coords, dtype=np.float32)
    positions = np.ascontiguousarray(positions, dtype=np.float32)
    weights = np.ascontiguousarray(weights, dtype=np.float32)
    features = np.ascontiguousarray(features, dtype=np.float32)
    ident, perm, pkinit = _host_consts()

    in_maps = [
        {
            "coords": coords[i * Q : (i + 1) * Q],
            "positions": positions,
            "weights": weights,
            "features": features,
            "ident": ident,
            "perm": perm,
            "pkinit": pkinit,
        }
        for i in range(NCORES)
    ]
    trace = bool(int(os.environ.get("KNN_TRACE", "0")))
    res = run_bass_kernel_spmd(_NC, in_maps, core_ids=list(range(NCORES)), trace=trace)
    LAST_RESULT = res
    return np.concatenate([res.results[i]["out"] for i in range(NCORES)], axis=0)



# revision 14
# speedup vs baseline: 39.2419x; 39.2419x over previous
"""Weighted-KNN (retrieval_knn) Trainium2 kernel.

Math (per query c, over N anchors):
    sq[n]   = ||c - p_n||^2 / (w_n^2 + eps)
    top-8 smallest sq -> softmax(-sq_k / TEMP) -> weighted sum of features.

Device strategy (per core, data-parallel over B across 8 cores):
  * y[q, n] = -sq[q,n]/TEMP computed on TensorE as a rank-5 inner product
    over centered coordinates (c' = c - 0.5, p' = p - 0.5):
        h_q = [||c'_q||^2, c'_q0, c'_q1, c'_q2, 1]
        g_n = (-1/TEMP) * inv_n * [1, -2p'_n0, -2p'_n1, -2p'_n2, ||p'_n||^2]
    The anchor axis is split into 4 groups handled by 4 concurrent
    row-tiled matmuls (tile_position=(32m, 0)); each group's 5 G rows and
    5 H rows live on disjoint 32-aligned partition lanes, so the four
    fp32 streams overlap on the PE array.
  * Packed top-8 trick: the PSUM->SBUF copy writes y as bf16 into the HIGH
    halves of persistent fp32 words whose LOW halves hold the (half-
    relative) anchor index, initialized once.  fp32 `max` (DVE top-8) on
    the packed words yields value ordering AND the index in one pass --
    no max_index scan.
  * Top-8 of each N/2 half (the union provably contains the true top-8:
    any true top-8 member has at most 7 better anchors anywhere).
  * The 16 candidates are re-scored EXACTLY from gathered [p', -inv/TEMP]
    rows (direct differences, no cancellation); top-8 selection + softmax
    run on the exact scores, so bf16/matmul rounding only perturbs
    candidates around global rank ~16 whose weights are negligible.
  * Feature rows fetched with gpsimd dma_gather; masked softmax-weighted
    sum on VectorE.
"""

import sys

if "/opt/trn_rl_repo" not in sys.path:
    sys.path.insert(0, "/opt/trn_rl_repo")

import numpy as np

import concourse.bacc as bacc
import concourse.bass as bass
import concourse.mybir as mybir
from concourse.bass import ts
from concourse.bass_utils import run_bass_kernel_spmd
from concourse.tile import TileContext

B, N, D, F = 65536, 16384, 3, 64
K = 8
BANDWIDTH = 0.05
TEMP = 2.0 * BANDWIDTH * BANDWIDTH  # 0.005
INV_TEMP = 1.0 / TEMP  # 200.0
EPS = 1e-8
NCORES = 8
Q = B // NCORES  # 8192 queries per core
P = 128
NT = Q // P  # 64 query tiles per core
CH = 512  # matmul free-dim chunk
NG = 4  # row-tiled matmul groups
NGN = N // NG  # 4096 anchors per group
NCHG = NGN // CH  # 8 chunk-steps
NH = N // NG  # 4096 anchors per quarter(=group)
NC = NG * K  # 32 candidates per query
NPK = 6  # packed-y buffer rotation depth
LOOP = 1  # in-NEFF repetitions of the whole tile loop (benchmarking)
STAGE = 99  # bench bisect: 1=mm+max8, 2=+perm/idxw, 3=+gathers, 99=full

FP = mybir.dt.float32
BF = mybir.dt.bfloat16
I32 = mybir.dt.int32


def _build_nc():
    nc = bacc.Bacc("TRN2", num_swdge_queues=2)
    coords = nc.declare_dram_parameter("coords", [Q, D], FP, isOutput=False)
    positions = nc.declare_dram_parameter("positions", [N, D], FP, isOutput=False)
    weights = nc.declare_dram_parameter("weights", [N], FP, isOutput=False)
    features = nc.declare_dram_parameter("features", [N, F], FP, isOutput=False)
    ident_in = nc.declare_dram_parameter("ident", [P, P], FP, isOutput=False)
    perm_in = nc.declare_dram_parameter("perm", [P, 8, P], FP, isOutput=False)
    pkinit_in = nc.declare_dram_parameter("pkinit", [P, NH], I32, isOutput=False)
    out = nc.declare_dram_parameter("out", [Q, F], FP, isOutput=True)

    # combined gather table: row n = [features(64) | p'_0 p'_1 p'_2 -inv/TEMP | pad]
    comb_hbm = nc.dram_tensor("comb_stage", [N, 2 * F], FP)

    with TileContext(nc) as tc:
        with (
            tc.tile_pool(name="const", bufs=1) as cpool,
            nc.gpsimd.register("nidx") as nidx_reg,
        ):
            nc.gpsimd.reg_mov(nidx_reg, P * K)

            ident = cpool.tile([P, P], FP)
            nc.sync.dma_start(ident[:], ident_in[:])
            pconst = cpool.tile([P, 8, P], FP)
            nc.sync.dma_start(pconst[:], perm_in[:])

            # G4[32m + r, j] = g_r[m*4096 + j]  (4 groups on partition lanes)
            G4 = cpool.tile([P, NGN], FP)

            # persistent packed-y buffers; low halves = half-relative idx
            pk = [
                cpool.tile([P, NH], FP, name=f"pk{i}", tag=f"pk{i}")
                for i in range(NPK)
            ]
            for i in range(NPK):
                nc.sync.dma_start(pk[i][:].bitcast(I32), pkinit_in[:])

            # ---------------- prep: build G and the rescore table ----------------
            with tc.tile_pool(name="prep", bufs=2) as pp:
                # anchors laid out n = 128*p + j
                pos_sb = pp.tile([P, P, D], FP)
                nc.sync.dma_start(
                    pos_sb[:], positions[:].rearrange("(p j) d -> p j d", p=P)
                )
                # center: p' = p - 0.5
                nc.vector.tensor_scalar_add(pos_sb[:], pos_sb[:], -0.5)
                w_sb = pp.tile([P, P], FP)
                nc.sync.dma_start(w_sb[:], weights[:].rearrange("(p j) -> p j", p=P))

                inv = pp.tile([P, P], FP)
                nc.vector.tensor_mul(inv[:], w_sb[:], w_sb[:])
                nc.vector.tensor_scalar_add(inv[:], inv[:], EPS)
                nc.vector.reciprocal(inv[:], inv[:])

                g0 = pp.tile([P, P], FP)
                nc.vector.tensor_scalar_mul(g0[:], inv[:], -INV_TEMP)

                gd = [
                    pp.tile([P, P], FP, tag=f"g{d + 1}", name=f"g{d + 1}")
                    for d in range(D)
                ]
                for d in range(D):
                    nc.vector.tensor_mul(gd[d][:], inv[:], pos_sb[:, :, d])
                    nc.vector.tensor_scalar_mul(gd[d][:], gd[d][:], 2.0 * INV_TEMP)

                pp2 = pp.tile([P, P], FP)
                tmp = pp.tile([P, P], FP)
                nc.vector.tensor_mul(pp2[:], pos_sb[:, :, 0], pos_sb[:, :, 0])
                nc.vector.tensor_mul(tmp[:], pos_sb[:, :, 1], pos_sb[:, :, 1])
                nc.vector.tensor_add(pp2[:], pp2[:], tmp[:])
                nc.vector.tensor_mul(tmp[:], pos_sb[:, :, 2], pos_sb[:, :, 2])
                nc.vector.tensor_add(pp2[:], pp2[:], tmp[:])
                g4c = pp.tile([P, P], FP)
                nc.vector.tensor_mul(g4c[:], g0[:], pp2[:])

                # scatter [128, 128] component tiles into G4 group lanes:
                # comp_r partitions [32m, 32m+32) hold n in [4096m, 4096m+4096)
                for r, comp in enumerate([g0, gd[0], gd[1], gd[2], g4c]):
                    for m in range(NG):
                        src = comp[32 * m : 32 * (m + 1), :]
                        dst = bass.AP(
                            G4[:].tensor,
                            (32 * m + r) * NGN,
                            [[NGN, 1], [P, 32], [1, P]],
                        )
                        nc.sync.dma_start(dst, src)

                # rescore table: interleave [p'0, p'1, p'2, g0] per anchor,
                # DMA'd (chunked) into the first 16 bytes of each 256B row.
                pwt = pp.tile([P, P, 4], FP)
                for f, comp in enumerate(
                    [pos_sb[:, :, 0], pos_sb[:, :, 1], pos_sb[:, :, 2], g0[:]]
                ):
                    nc.vector.tensor_copy(pwt[:, :, f], comp)
                pw_rows = comb_hbm[:, F : F + 4].rearrange("(p j) f -> p j f", p=P)
                for ck in range(8):
                    nc.sync.dma_start(
                        pw_rows[ts(ck, 16), :, :], pwt[ts(ck, 16), :, :]
                    )
                # features -> comb rows (HBM->HBM, chunked)
                feat_rows = comb_hbm[:, 0:F].rearrange("(a n) f -> a n f", a=8)
                src_rows = features[:].rearrange("(a n) f -> a n f", a=8)
                for ck in range(8):
                    nc.sync.dma_start(feat_rows[ck, :, :], src_rows[ck, :, :])

            # ---------------- main loop over query tiles ----------------
            with (
                tc.tile_pool(name="mm_ps", bufs=6, space="PSUM") as pspool,
                tc.tile_pool(name="ht_ps", bufs=2, space="PSUM") as htpool,
                tc.tile_pool(name="hs", bufs=4) as hpool,
                tc.tile_pool(name="sm", bufs=6) as sm,
                tc.tile_pool(name="g8", bufs=3) as gpool,
            ):
                for tl in range(NT * LOOP):
                    t = tl % NT
                    # --- per-tile H build, replicated to 4 group lanes ---
                    ct = hpool.tile([P, D], FP, tag="ct")
                    nc.sync.dma_start(ct[:], coords[ts(t, P), :])
                    nc.vector.tensor_scalar_add(ct[:], ct[:], -0.5)
                    nct = hpool.tile([P, D], FP, tag="nct")
                    nc.vector.tensor_scalar_mul(nct[:], ct[:], -1.0)
                    cc = hpool.tile([P, D], FP, tag="cc")
                    nc.vector.tensor_mul(cc[:], ct[:], ct[:])
                    hsrc = hpool.tile([P, 5], FP, tag="hsrc")
                    nc.vector.reduce_sum(
                        out=hsrc[:, 0:1], in_=cc[:], axis=mybir.AxisListType.X
                    )
                    nc.vector.tensor_copy(hsrc[:, 1:4], ct[:])
                    nc.vector.memset(hsrc[:, 4:5], 1.0)
                    psT = htpool.tile([P, P], FP, tag="htmp")
                    nc.tensor.transpose(psT[:5, :], hsrc[:], ident[:])
                    hT4 = hpool.tile([P, P], FP, tag="hT4")
                    for m in range(NG):
                        nc.scalar.copy(hT4[32 * m : 32 * m + 5, :], psT[:5, :])

                    # --- distances: 4 concurrent row-tiled matmuls/step ---
                    pkb = [pk[(NG * tl + m) % NPK] for m in range(NG)]
                    for c in range(NCHG):
                        for m in range(NG):
                            ps = pspool.tile(
                                [P, CH], FP, tag="ps", name=f"ps{tl}_{c}_{m}"
                            )
                            nc.tensor.matmul(
                                ps[:],
                                hT4[32 * m : 32 * m + 5, :],
                                G4[32 * m : 32 * m + 5, ts(c, CH)],
                                start=True,
                                stop=True,
                                tile_position=(32 * m, 0),
                            )
                            # bf16 y into high halves of group m's words
                            hi = bass.AP(
                                pkb[m][:].bitcast(BF).tensor,
                                2 * CH * c + 1,
                                [[2 * NH, P], [2, CH]],
                            )
                            nc.scalar.copy(hi, ps[:])

                    # --- packed top-8 per quarter; extract indices ---
                    idx32 = sm.tile([P, NC], I32, tag="idx32")
                    for m in range(NG):
                        v8p = sm.tile([P, K], FP, tag="v8p", name=f"v8p_{tl}_{m}")
                        nc.vector.max(v8p[:], pkb[m][:])
                        nc.vector.tensor_scalar(
                            idx32[:, ts(m, K)],
                            v8p[:].bitcast(I32),
                            65535,
                            None,
                            op0=mybir.AluOpType.bitwise_and,
                        )
                        if m:
                            nc.vector.tensor_scalar_add(
                                idx32[:, ts(m, K)], idx32[:, ts(m, K)], float(m * NH)
                            )

                    if STAGE == 1:
                        dump = sm.tile([P, F], FP, tag="dump", name=f"dump{tl}")
                        nc.vector.tensor_copy(dump[:, 0:NC], idx32[:])
                        nc.sync.dma_start(out[ts(t, P), :], dump[:])
                        continue

                    # --- wrapped int16 idx layout for dma_gather:
                    # idxw[16g+p, 8k+u] = idx[16u+p, k] via 8 permutation
                    # matmuls (perm_u[q, p'] = 1 iff q == 16u + p'%16).
                    idxf = sm.tile([P, NC], FP, tag="idxf")
                    nc.vector.tensor_copy(idxf[:], idx32[:])
                    psI = htpool.tile([P, 8, NC], FP, tag="htmp", name=f"psI_{tl}")
                    for u in range(8):
                        nc.tensor.matmul(
                            psI[:, u, :],
                            pconst[:, u, :],
                            idxf[:],
                            start=True,
                            stop=True,
                        )
                    idxw = sm.tile([P, NC * 8], mybir.dt.int16, tag="idxw")
                    idxw_uk = bass.AP(
                        idxw[:].tensor, 0, [[NC * 8, P], [1, 8], [8, NC]]
                    )
                    nc.vector.tensor_copy(idxw_uk, psI[:])

                    if STAGE == 2:
                        dump = sm.tile([P, F], FP, tag="dump", name=f"dump{tl}")
                        nc.vector.tensor_copy(dump[:], idxw[:, 0:128].bitcast(FP))
                        nc.sync.dma_start(out[ts(t, P), :], dump[:])
                        continue

                    # --- gather candidate rescore rows + feature rows ---
                    cg = gpool.tile([P, NC, 2 * F], FP, tag="cg")
                    for m in range(NG):
                        isl = idxw[:, m * K * 8 : (m + 1) * K * 8]
                        nc.gpsimd.dma_gather(
                            cg[:, m * K : (m + 1) * K, :],
                            comb_hbm[:],
                            isl,
                            P * K,
                            nidx_reg,
                            2 * F,
                            queue_num=m % 2,
                        )
                    g32 = cg[:, :, 0:F]
                    pwg = cg[:, :, F : 2 * F]

                    if STAGE == 3:
                        dump = sm.tile([P, F], FP, tag="dump", name=f"dump{tl}")
                        nc.vector.tensor_add(dump[:], cg[:, 0, 0:F], cg[:, 0, F:2*F])
                        nc.sync.dma_start(out[ts(t, P), :], dump[:])
                        continue

                    # --- exact rescore: y32 = sum_d (p'_d - c'_d)^2 * g0 ---
                    sqd = [
                        sm.tile([P, NC], FP, tag=f"sqd{d}", name=f"sqd{d}")
                        for d in range(D)
                    ]
                    for d in range(D):
                        nc.scalar.activation(
                            sqd[d][:],
                            pwg[:, :, d],
                            mybir.ActivationFunctionType.Square,
                            bias=nct[:, d : d + 1],
                            scale=1.0,
                        )
                    nc.vector.tensor_add(sqd[0][:], sqd[0][:], sqd[1][:])
                    nc.vector.tensor_add(sqd[0][:], sqd[0][:], sqd[2][:])
                    y16 = sm.tile([P, NC], FP, tag="y16")
                    nc.vector.tensor_mul(y16[:], sqd[0][:], pwg[:, :, 3])

                    # --- exact top-8 + masked softmax over 16 candidates ---
                    v8x = sm.tile([P, K], FP, tag="v8x")
                    nc.vector.max(v8x[:], y16[:])
                    nv1 = sm.tile([P, 1], FP, tag="nv1")
                    nc.vector.tensor_scalar_mul(nv1[:], v8x[:, 0:1], -1.0)
                    e16 = sm.tile([P, NC], FP, tag="e16")
                    nc.scalar.activation(
                        e16[:],
                        y16[:],
                        mybir.ActivationFunctionType.Exp,
                        bias=nv1[:],
                        scale=1.0,
                    )
                    m16 = sm.tile([P, NC], FP, tag="m16")
                    nc.vector.tensor_scalar(
                        m16[:],
                        y16[:],
                        v8x[:, K - 1 : K],
                        None,
                        op0=mybir.AluOpType.is_ge,
                    )
                    ew = sm.tile([P, NC], FP, tag="ew")
                    nc.vector.tensor_mul(ew[:], e16[:], m16[:])
                    ssum = sm.tile([P, 1], FP, tag="ssum")
                    nc.vector.reduce_sum(
                        out=ssum[:], in_=ew[:], axis=mybir.AxisListType.X
                    )
                    rs = sm.tile([P, 1], FP, tag="rs")
                    nc.vector.reciprocal(rs[:], ssum[:])

                    # --- weighted sum of candidate features ---
                    nc.vector.tensor_mul(
                        g32, g32, ew[:].to_broadcast([P, NC, F])
                    )
                    half = NC
                    while half > 1:
                        half //= 2
                        nc.vector.tensor_add(
                            cg[:, 0:half, 0:F],
                            cg[:, 0:half, 0:F],
                            cg[:, half : 2 * half, 0:F],
                        )
                    ot = gpool.tile([P, F], FP, tag="ot")
                    nc.vector.tensor_scalar_mul(ot[:], cg[:, 0, 0:F], rs[:])

                    nc.sync.dma_start(out[ts(t, P), :], ot[:])

    nc.compile()
    return nc


_NC = None
LAST_RESULT = None


def _host_consts():
    ident = np.eye(P, dtype=np.float32)
    perm = np.zeros((P, 8, P), dtype=np.float32)
    for u in range(8):
        for p16 in range(16):
            perm[16 * u + p16, u, p16::16] = 1.0
    pkinit = np.tile(np.arange(NH, dtype=np.int32), (P, 1))
    return ident, perm, pkinit


def kernel(coords, positions, weights, features):
    global _NC, LAST_RESULT
    import os

    if _NC is None:
        _NC = _build_nc()

    coords = np.ascontiguousarray(coords, dtype=np.float32)
    positions = np.ascontiguousarray(positions, dtype=np.float32)
    weights = np.ascontiguousarray(weights, dtype=np.float32)
    features = np.ascontiguousarray(features, dtype=np.float32)
    ident, perm, pkinit = _host_consts()

    in_maps = [
        {
            "coords": coords[i * Q : (i + 1) * Q],
            "positions": positions,
            "weights": weights,
            "features": features,
            "ident": ident,
            "perm": perm,
            "pkinit": pkinit,
        }
        for i in range(NCORES)
    ]
    trace = bool(int(os.environ.get("KNN_TRACE", "0")))
    res = run_bass_kernel_spmd(_NC, in_maps, core_ids=list(range(NCORES)), trace=trace)
    LAST_RESULT = res
    return np.concatenate([res.results[i]["out"] for i in range(NCORES)], axis=0)

